# revision 31
# baseline (speedup 1.0000x reference)
"""Trainium2 Bass kernel for nn_CrossChannelAttention.

Reference computation (per batch b, pixel p, with C=128 channels, NUMS=16
groups of HEADS=8 channels, OUT=256):
    fm[g,p]  = relu(sum_h W1[g,h] * x[8g+h, p] + b1[g])          # [16, P]
    feat[(g,d), p] = fm[g,p] * x[d,p]                            # [2048, P]
    out[o,p] = sum_c W2[o,c] * feat[c,p] + b2[o]                 # [256, P]

Strategy: data-parallel over batch B=8 across the 8 NeuronCores (one image
per core, params replicated).  Per core the PE-bound floor is 256 bf16
matmuls [K=128,M=128,N=512] ~= 55us; everything else must hide under it.

v2 changes vs the 92us baseline (which had a 24us pipeline-fill prologue,
cold-clock matmuls and an 8us drain tail):
  - PE warmup: DVE-memset scratch + dummy N=512 matmuls issued from ~6us so
    the HAM clock gate is at 2.4 GHz before the first real matmul.
  - HWDGE issue streams reordered: sync ring does pure loads first
    (w1s, x0 half, w2t chunk0, x1..x3, w2t chunks) then the bulk fm-row
    broadcasts in consumption order; scalar ring does biases, the other x0
    half, per-half fm DRAM writes (immediately after each RELU), the k=0
    bootstrap broadcasts, PSUM drains and output stores.  No load ever
    queues behind a compute-dependent DMA.
  - gpsimd partition_broadcast reads fm rows directly from SBUF (no DRAM
    round-trip) for bootstrap + relief units, in [128,512] halves.
  - fm tiles, w2t chunks and x tiles are separate tiles per dependency unit
    so Tile never invents whole-tile false dependencies.
  - tail: last-k PSUM drains split scalar/vector, stores split scalar/sync.
Accuracy: bf16 matmuls with fp32 PSUM accumulation; rel err ~4e-3.
"""

import numpy as np
import ml_dtypes

import concourse.bacc as bacc
import concourse.tile as tile
from concourse import mybir
from concourse.bass_utils import run_bass_kernel_spmd

F32 = mybir.dt.float32
BF16 = mybir.dt.bfloat16

B, C, H, W = 8, 128, 64, 64
NUMS, HEADS, OUT = 16, 8, 256
P = H * W          # 4096 pixels per image
PB = 512           # pixel block (one PSUM bank of fp32)
GRP = 1024         # broadcast chunk (2 pixel blocks)
NGRP = P // GRP    # 4 broadcast groups
N_CORES = 8
LOOKAHEAD = 10     # broadcast/feat pipeline depth in (g,k) units
N_WARMUP = 6       # dummy matmuls to warm the PE clock gate

# g=0 units of k>=1 get their rep eagerly via gpsimd partition_broadcast
# straight from SBUF right after each k's RELU (only partition 0 is legal
# as a gpsimd source, which is exactly row g=0); the feat multiply stays at
# the consumption position.  This pre-produces every k-transition ft so the
# PE never stalls at g=0, and gpsimd's ~3.5us dispatch latency is hidden.
GPS_UNITS = {(0, 1), (0, 2), (0, 3)}
# k=0 bootstrap units (0,0),(1,0) are produced entirely on-chip: a K=1
# matmul against a ones-row broadcasts the fm row (duplicated at partition
# 0/32 via the widened w1s) into PSUM, scalar copies PSUM->SBUF, DVE
# multiplies.  Every DMA-completion hop costs ~1.3us of semaphore latency,
# so the engine-only chain starts the mains ~8us earlier.
PE_BOOT = ((0, 0), (1, 32))   # (g, fm row in the widened layout)

_CACHE = {}


def _build():
    nc = bacc.Bacc("TRN2", target_bir_lowering=False, debug=False,
                   num_devices=N_CORES)

    x_d = nc.dram_tensor("x", [C, P], BF16, kind="ExternalInput")
    # w1s widened to M=128: cols 0-15 = groups 0-15, col 32 = group 1
    # duplicate, col 64/96 spare duplicates (for the PE-broadcast bootstrap)
    w1s_d = nc.dram_tensor("w1s", [C, C], BF16, kind="ExternalInput")
    w2t_d = nc.dram_tensor("w2t", [C, NUMS * OUT], BF16, kind="ExternalInput")
    b1_d = nc.dram_tensor("b1c", [C, 1], F32, kind="ExternalInput")
    b2_d = nc.dram_tensor("b2c", [C, 2], F32, kind="ExternalInput")
    # bf16 output halves the 4MB store traffic; ~0.2% extra error vs the
    # 2e-2 budget (host converts back to fp32)
    out_d = nc.dram_tensor("out", [OUT, P], BF16, kind="ExternalOutput")

    relu = mybir.ActivationFunctionType.Relu
    ident = mybir.ActivationFunctionType.Identity
    copyf = mybir.ActivationFunctionType.Copy
    mult = mybir.AluOpType.mult

    with tile.TileContext(nc) as tc:
        with (
            tc.tile_pool(name="const", bufs=1) as cpool,
            tc.tile_pool(name="xbp", bufs=4) as xbp,
            tc.tile_pool(name="fmp", bufs=8) as fmp,
            tc.tile_pool(name="repp", bufs=12) as repp,
            tc.tile_pool(name="reph", bufs=10) as rephp,
            tc.tile_pool(name="feat", bufs=LOOKAHEAD + 4) as featp,
            tc.tile_pool(name="feath", bufs=8) as fethp,
            tc.tile_pool(name="osb", bufs=4) as osb,
            tc.tile_pool(name="ps", bufs=8, space="PSUM") as ps,
            tc.tile_pool(name="dr", bufs=4, space="DRAM") as drp,
        ):
            # ---- PE warmup: gpsimd-memset scratch, N=512 dummy matmuls ----
            # bridges the PE from engine-start (~6.5us) to x2[0] arrival so
            # the HAM clock gate is warm for the fm matmuls and mains
            scratch = cpool.tile([C, C + PB], BF16)
            nc.gpsimd.memset(scratch[:], 0.0)
            ones_t = cpool.tile([2 * 32 + 1, C], BF16)
            nc.gpsimd.memset(ones_t[:], 1.0)
            ps_w = ps.tile([C, PB], F32, tag="ps", name="ps_warm")
            for i in range(N_WARMUP):
                nc.tensor.matmul(ps_w[:], scratch[:, 0:C],
                                 scratch[:, C:C + PB], start=True, stop=True)

            # ---- input loads: pure-load prefix (sync ring) ----
            w1s_t = cpool.tile([C, C], BF16)
            nc.sync.dma_start(w1s_t[:], w1s_d[:])
            b1_t = cpool.tile([C, 1], F32)
            nc.scalar.dma_start(b1_t[:], b1_d[:])
            b2_t = cpool.tile([C, 2], F32)
            nc.scalar.dma_start(b2_t[:], b2_d[:])

            x2s = []
            for k in range(NGRP):
                x2s.append(xbp.tile([C, GRP], BF16, tag=f"x2_{k}",
                                    name=f"x2_{k}"))
            # x2[0] halves split across the two HWDGE rings for latency
            nc.sync.dma_start(x2s[0][:, 0:PB], x_d[:, 0:PB])
            nc.scalar.dma_start(x2s[0][:, PB:GRP], x_d[:, PB:GRP])
            # w2t in 2 chunks of 8 groups: c0 needed by g=0, c1 by g=8
            w2t_c = [cpool.tile([C, 8 * 2 * C], BF16, name=f"w2t_c{j}")
                     for j in range(2)]
            nc.sync.dma_start(w2t_c[0][:], w2t_d[:, 0:2048])
            for k in range(1, NGRP):
                gx = slice(k * GRP, (k + 1) * GRP)
                nc.sync.dma_start(x2s[k][:], x_d[:, gx])

            # ---- fm: matmul + relu per (k, half) ----
            # fmh[(k, h)] : [NUMS, PB] SBUF tiles (separate => fine-grained deps)
            # g=0 rep halves via gpsimd partition_broadcast immediately after
            # each RELU (SBUF source, partition 0); their feat MULTIPLYs stay
            # at the consumption position so the DVE FIFO is never blocked --
            # except (0,0) whose MULTIPLY bootstraps the whole pipeline.
            fmh = {}
            gps_reps = {}   # (k, h) -> rep tile for g=0
            fm_drs = [drp.tile([NUMS, GRP], BF16, tag=f"fmdr{k}",
                               name=f"fmdr{k}") for k in range(NGRP)]
            fts = {}   # (g,k) -> ("full", tile) | ("half", [t0, t1])

            def emit_fmw(k):
                # one DMA per half keeps the source tiles separate
                for h in range(2):
                    hx = slice(h * PB, (h + 1) * PB)
                    nc.sync.dma_start(fm_drs[k][:, hx],
                                      fmh[(k, h)][0:NUMS, :])

            # k=0: fm matmul produces the widened [C, PB] fm (rows 0-15 =
            # groups, row 32 = group-1 duplicate), then the PE-broadcast
            # bootstrap for (0,0) and (1,0): K=1 matmul against a ones row
            # fans the fm row across all 128 PSUM partitions, scalar copies
            # it to SBUF, DVE multiplies.  No DMA on the critical chain.
            for h in range(2):
                hx = slice(h * PB, (h + 1) * PB)
                t = fmp.tile([C, PB], BF16, tag=f"fm0_{h}", name=f"fm0_{h}")
                fmh[(0, h)] = t
                ps_fm = ps.tile([C, PB], F32, tag="ps", name=f"psfm0_{h}")
                nc.tensor.matmul(ps_fm[:], w1s_t[:], x2s[0][:, hx],
                                 start=True, stop=True)
                nc.scalar.activation(t[:], ps_fm[:], relu, bias=b1_t[:])
            emit_fmw(0)
            for (gb, row) in PE_BOOT:
                halves = []
                for h in range(2):
                    hx = slice(h * PB, (h + 1) * PB)
                    ps_bc = ps.tile([C, PB], F32, tag="ps",
                                    name=f"psbc{gb}_{h}")
                    nc.tensor.matmul(ps_bc[:], ones_t[row:row + 1, :],
                                     fmh[(0, h)][row:row + 1, :],
                                     start=True, stop=True)
                    rep = rephp.tile([C, PB], BF16, tag="reph",
                                     name=f"rb{gb}_{h}")
                    nc.scalar.activation(rep[:], ps_bc[:], copyf)
                    ft = fethp.tile([C, PB], BF16, tag="fth",
                                    name=f"fthb{gb}_{h}")
                    nc.vector.tensor_tensor(ft[:], x2s[0][:, hx], rep[:],
                                            op=mult)
                    halves.append(ft)
                fts[(gb, 0)] = ("half", halves)

            # k>=1: fm + eager gpsimd partition_broadcast of row g=0 (feat
            # multiply stays at the consumption position)
            for k in range(1, NGRP):
                for h in range(2):
                    hx = slice(h * PB, (h + 1) * PB)
                    t = fmp.tile([NUMS, PB], BF16, tag=f"fm{k}_{h}",
                                 name=f"fm{k}_{h}")
                    fmh[(k, h)] = t
                    ps_fm = ps.tile([NUMS, PB], F32, tag="ps",
                                    name=f"psfm{k}_{h}")
                    nc.tensor.matmul(ps_fm[:], w1s_t[:, 0:NUMS],
                                     x2s[k][:, hx], start=True, stop=True)
                    nc.scalar.activation(t[:], ps_fm[:], relu,
                                         bias=b1_t[0:NUMS, :])
                    rep = rephp.tile([C, PB], BF16, tag="reph",
                                     name=f"rh0_{k}_{h}")
                    nc.gpsimd.partition_broadcast(rep[:], t[0:1, :])
                    gps_reps[(k, h)] = rep

            # ---- broadcast + feat producers ----
            def emit_unit(g, k):
                if (g, k) in fts:
                    return   # bootstrap unit already produced
                if (g, k) in GPS_UNITS:
                    halves = []
                    for h in range(2):
                        ft = fethp.tile([C, PB], BF16, tag="fth",
                                        name=f"fth{g}_{k}_{h}")
                        hx = slice(h * PB, (h + 1) * PB)
                        nc.vector.tensor_tensor(ft[:], x2s[k][:, hx],
                                                gps_reps[(k, h)][:], op=mult)
                        halves.append(ft)
                    fts[(g, k)] = ("half", halves)
                else:
                    rep = repp.tile([C, GRP], BF16, tag="rep",
                                    name=f"rep{g}_{k}")
                    nc.sync.dma_start(
                        rep[:], fm_drs[k][g:g + 1, :].broadcast_to((C, GRP)))
                    ft = featp.tile([C, GRP], BF16, tag="ft",
                                    name=f"ft{g}_{k}")
                    nc.vector.tensor_tensor(ft[:], x2s[k][:], rep[:], op=mult)
                    fts[(g, k)] = ("full", ft)

            todo = [(g, k) for k in range(NGRP) for g in range(NUMS)]
            for i in range(LOOKAHEAD):
                emit_unit(*todo[i])
                if i == 2:
                    nc.sync.dma_start(w2t_c[1][:], w2t_d[:, 2048:4096])
                if i == 6:
                    emit_fmw(1)

            # ---- main matmuls + drains ----
            def w2blk(g, oc):
                j, r = divmod(g, 8)
                cx = slice((r * 2 + oc) * C, (r * 2 + oc + 1) * C)
                return w2t_c[j][:, cx]

            pso = {}
            for i, (g, k) in enumerate(todo):
                if i + LOOKAHEAD < len(todo):
                    gn, kn = todo[i + LOOKAHEAD]
                    if (gn, kn) == (8, 1):
                        emit_fmw(2)
                    if (gn, kn) == (8, 2):
                        emit_fmw(3)
                    emit_unit(gn, kn)
                kind, ft = fts.pop((g, k))
                if g == 0:
                    for pbb in (2 * k, 2 * k + 1):
                        for oc in range(2):
                            pso[(pbb, oc)] = ps.tile([C, PB], F32, tag="ps",
                                                     name=f"pso{pbb}_{oc}")
                for h in range(2):
                    pb = 2 * k + h
                    rhs = (ft[h][:] if kind == "half"
                           else ft[:, h * PB:(h + 1) * PB])
                    for oc in range(2):
                        nc.tensor.matmul(pso[(pb, oc)][:], w2blk(g, oc), rhs,
                                         start=(g == 0), stop=(g == NUMS - 1))
                if g == NUMS - 1:
                    last = (k == NGRP - 1)
                    for pbb in (2 * k, 2 * k + 1):
                        px = slice(pbb * PB, (pbb + 1) * PB)
                        o0 = osb.tile([C, PB], BF16, tag="osb",
                                      name=f"o0_{pbb}")
                        o1 = osb.tile([C, PB], BF16, tag="osb",
                                      name=f"o1_{pbb}")
                        nc.scalar.activation(o0[:], pso.pop((pbb, 0))[:],
                                             ident, bias=b2_t[:, 0:1])
                        if last:
                            nc.vector.tensor_scalar_add(
                                o1[:], pso.pop((pbb, 1))[:], b2_t[:, 1:2])
                        else:
                            nc.scalar.activation(o1[:], pso.pop((pbb, 1))[:],
                                                 ident, bias=b2_t[:, 1:2])
                        nc.scalar.dma_start(out_d[0:C, px], o0[:])
                        if last:
                            nc.sync.dma_start(out_d[C:OUT, px], o1[:])
                        else:
                            nc.scalar.dma_start(out_d[C:OUT, px], o1[:])

    nc.compile()
    return nc


def _prep_params(W1, b1, W2, b2):
    bf = ml_dtypes.bfloat16
    # w1s[c, g] = W1[g, c - 8g] for 8g <= c < 8(g+1), else 0; widened to
    # [C, C] with col 32 = col 1 (group-1 duplicate for PE-boot broadcast)
    w1s = np.zeros((C, C), dtype=bf)
    for g in range(NUMS):
        w1s[g * HEADS:(g + 1) * HEADS, g] = W1[g].astype(bf)
    w1s[:, 32] = w1s[:, 1]
    # w2t[k, (g*2+oc)*128 + m] = W2[oc*128 + m, g*128 + k]
    w2t = (
        np.asarray(W2, dtype=np.float32)
        .reshape(2, C, NUMS, C)          # [oc, m, g, k]
        .transpose(3, 2, 0, 1)           # [k, g, oc, m]
        .reshape(C, NUMS * OUT)
        .astype(bf)
    )
    b1c = np.zeros((C, 1), dtype=np.float32)
    b1c[0:NUMS, 0] = np.asarray(b1, dtype=np.float32)
    b1c[32, 0] = b1c[1, 0]
    b2c = np.asarray(b2, dtype=np.float32).reshape(2, C).T.copy()
    return w1s, w2t, b1c, b2c


def kernel(x, W1, b1, W2, b2, _trace=False, _trace_kwargs=None):
    if "nc" not in _CACHE:
        _CACHE["nc"] = _build()
    nc = _CACHE["nc"]

    w1s, w2t, b1c, b2c = _prep_params(W1, b1, W2, b2)
    xs = np.ascontiguousarray(
        np.asarray(x, dtype=np.float32).reshape(B, C, P).astype(ml_dtypes.bfloat16))
    in_maps = [
        {"x": xs[b_], "w1s": w1s, "w2t": w2t, "b1c": b1c, "b2c": b2c}
        for b_ in range(N_CORES)
    ]
    kwargs = {}
    if _trace:
        kwargs["trace"] = True
        kwargs.update(_trace_kwargs or {})
    res = run_bass_kernel_spmd(nc, in_maps, core_ids=list(range(N_CORES)),
                               **kwargs)
    out = np.stack([np.asarray(res.results[b_]["out"], dtype=np.float32)
                    for b_ in range(N_CORES)])
    out = out.reshape(B, OUT, H, W)
    if _trace:
        _CACHE["last_result"] = res
    return out
